# revision 39
# baseline (speedup 1.0000x reference)
"""ArDCA pseudo-likelihood loss on 8 Trainium2 NeuronCores.

Math (reference): for samples X (M,L) over alphabet Q with weights W,
    pair[m,i,a] = sum_{j<i} J[i,j,a,X[m,j]]
    logits = h_pos + pair ;  loss = -sum_{m,i} W[m]*log_softmax(logits)[gold]
                              + lam_h*|h|^2 + lam_j*|tril(J)|^2

Strategy: data-parallel over M (1024 samples/core).  The one-hot einsum is a
dense TensorEngine matmul: out[m, (i,a)] += onehotT[(j,b), m].T @ J[(j,b),(i,a)]
with K = L*Q = 5376 contraction packed 6 j-positions per 128-row K-tile and the
strict lower-triangle (j<i) skipped at tile granularity.  J (tril-masked,
scaled by 64, fp8-e4m3, column-packed) stays SBUF-resident and streams through
the PE as fp8 DoubleRow pairs (two K-tiles per matmul, 2 weights/cell); h_pos
rides in as a bias row of K-tile 0 against an all-ones row of the one-hot.
Each m-tile runs as three 4+4+3-psum-bank segs so a bank's reacquisition
trails its release by ~2 segs of matmul work.  Epilogue per (m-tile,
i-block): exp on ScalarE (bf16 out, scale=1/64 folded in, sole psum reader
so banks release at ACT pace), gold extraction in exp domain (e*onehot with
the host-built, DMA-streamed one-hot has one nonzero per 21-seg), one merged
segment-reduce on the DVE covering denominators and golds, then two
Ln+accum per m-tile (batched at the end -- Exp and Ln don't share an ACT
table set).  regJ/regH are float64 on the host.  Each core emits a (128,1)
partial NLL column; the host sums them and adds the regularizers.

Two post-scheduling BIR passes matter: a global NO_SYNC chain pins the PE
stream to emission order (the tile scheduler otherwise orders matmuls
i-block-major, one weight reload per matmul), and _prune_ldweights dedups
the 256-column DoubleRow LDWEIGHTS within each same-lhs group (1056 -> 368
weight loads, ~60us of TensorE time).

Known landmines on this stack (found the hard way): tensor_tensor_reduce and
any fp8 operand on the VectorEngine hard-crash the device; engine APs must
start at 32-aligned partitions; interleaving Exp/Ln per-tile thrashes ACT
table sets (~1.3us per reload); walrus --enable-ldw-opt crashes codegen on
DoubleRow; post-hoc PE stream reordering (see _repack_pe_order, unused)
deadlocks the device -- order must be imposed via dependencies so the
scheduler's bank/slot bookkeeping stays consistent.
"""

import os
import sys

import numpy as np
import ml_dtypes

try:
    import concourse.bass as bass  # noqa: F401
except ImportError:  # pragma: no cover
    sys.path.insert(0, "/opt/trn_rl_repo")

import concourse.bass as bass
import concourse.mybir as mybir
import concourse.tile as tile
from concourse import bacc
from concourse import bass_utils as _bu
from concourse.bass_utils import run_bass_kernel_spmd



# ---------------------------------------------------------------- constants
M, L, Q = 8192, 256, 21
LAMBDA_H = 1e-06
LAMBDA_J = 0.0001

NCORES = 8
MC = M // NCORES        # 1024 samples per core
MT = MC // 128          # 8 m-tiles per core
LQ = L * Q              # 5376

JPK = 6                 # j-positions per K-tile (6*21=126 <= 128)
KT = (L + JPK - 1) // JPK   # 43 K-tiles
IB = 24                 # i-positions per i-block (24*21=504 <= 512 psum bank)
NIB = (L + IB - 1) // IB    # 11 i-blocks (10 of 24 + 1 of 16)
IB_N = [min(IB, L - IB * b) * Q for b in range(NIB)]  # 504 .. 336
SCALE = 64.0            # fp8 pre-scale on J / h

FP8 = ml_dtypes.float8_e4m3
BF16 = ml_dtypes.bfloat16

# first i-block each K-tile contributes to: need some i in block with i > 6*kt
BMIN = [(JPK * kt + 1) // IB for kt in range(KT)]
assert all(BMIN[kt] == kt // 4 for kt in range(KT))
# last K-tile contributing to i-block b:  j <= i_max-1 = min(IB*(b+1),L)-2
LASTKT = [min(KT - 1, (IB * (b + 1) - 2) // JPK) for b in range(NIB)]
# packed J column widths / offsets (per partition, in elements).  Widths are
# padded to 16 so the DoubleRow middle-dim step (= JW of the even K-tile of a
# pair) satisfies the ISA's step%16 requirement.
REALW = [LQ - 504 * BMIN[kt] for kt in range(KT)]
JW = [(w + 15) // 16 * 16 for w in REALW]
JOFS = np.concatenate([[0], np.cumsum(JW)]).astype(int)
TOTW = int(JOFS[-1])
NPAIR = KT // 2  # 21 DoubleRow pairs; K-tile 42 runs as a plain matmul

WAVES = [(0, 8), (8, NIB)]  # i-block ranges sharing the 8 psum banks

_DT = mybir.dt

# PE engine type for the LDWEIGHTS prune below (EngineType.PE)
_PE = mybir.EngineType.PE


def _repack_pe_order(nc):
    """Re-serialize each block with the PE stream in emission order.

    The tile scheduler orders matmuls by simulated readiness, which in
    steady state degenerates to i-block-major order across lhs groups --
    forcing a 256-column LDWEIGHTS reload before nearly every matmul.  The
    emission order (pair-major within each 4-bank seg) keeps same-lhs
    matmuls adjacent and is provably safe: per-bank accumulation chains
    stay ascending, and a seg's start=True matmuls only wait on exps of
    two segs earlier.  Rebuild the block as a dependency-respecting
    topological merge of per-engine streams, with the PE stream replaced
    by (LDWEIGHTS, MATMUL) units sorted by matmul emission id.  Runs
    before nc.compile(), so semaphores are generated from the new order.
    """
    for bb in nc.m.functions[0].blocks:
        insts = list(bb.instructions)
        # split into segments at barrier instructions (branches & friends)
        segments = []
        cur = []
        barriers = []
        for i in insts:
            tn = type(i).__name__
            if tn in ("InstLdweights", "InstMatmult") or getattr(
                i, "engine", None
            ) is not None and tn not in (
                "InstUnconditionalBranch",
                "InstCompareAndBranch",
                "InstBranchHint",
            ):
                cur.append(i)
            else:
                segments.append(cur)
                barriers.append(i)
                cur = []
        segments.append(cur)

        out = []
        for seg_idx, seg in enumerate(segments):
            n_mm = sum(1 for i in seg if type(i).__name__ == "InstMatmult")
            if n_mm < 2:
                out.extend(seg)
            else:
                out.extend(_topo_merge_pe_emission(seg))
            if seg_idx < len(barriers):
                out.append(barriers[seg_idx])
        assert len(out) == len(insts)
        bb.instructions.clear()
        bb.instructions.extend(out)


def _topo_merge_pe_emission(seg):
    from collections import defaultdict

    streams = defaultdict(list)   # engine -> instruction stream
    pe_units = []
    pending_ldw = None
    for i in seg:
        tn = type(i).__name__
        if tn == "InstLdweights":
            assert pending_ldw is None
            pending_ldw = i
        elif tn == "InstMatmult":
            unit = ([pending_ldw] if pending_ldw is not None else []) + [i]
            pending_ldw = None
            pe_units.append((int(i.name.split("-")[1]), unit))
        else:
            streams[i.engine].append([i])
    assert pending_ldw is None
    pe_units.sort(key=lambda u: u[0])
    streams[_PE] = [u for _, u in pe_units]

    local = {
        i.name
        for units in streams.values()
        for unit in units
        for i in unit
    }
    emitted = set()
    heads = {e: 0 for e in streams}
    result = []
    total = sum(len(u) for us in streams.values() for u in us)
    while len(result) < total:
        progress = False
        for e, units in streams.items():
            while heads[e] < len(units):
                unit = units[heads[e]]
                deps = set()
                names = {i.name for i in unit}
                for i in unit:
                    deps |= set(i.sync_dependency_set_copy())
                    deps |= set(i.nosync_dependency_set_copy())
                deps -= names
                if (deps & local) - emitted:
                    break
                result.extend(unit)
                emitted |= names
                heads[e] += 1
                progress = True
        if not progress:
            raise RuntimeError("topo merge stalled (dependency cycle?)")
    return result


def _prune_ldweights(nc):
    """Remove InstLdweights that reload the identical stationary operand.

    The tile legalizer splits every fp8 matmul into LDWEIGHTS+MATMUL; with
    DoubleRow the 256-column weight load (~200ns) costs ~2x the ~105ns matmul
    stream, and 10 of every 11 loads here re-load the same one-hot tile (the
    inner i-block loop shares lhs).  Walrus's own dedup (--enable-ldw-opt)
    crashes codegen on DoubleRow, so dedup at the BIR level: scan each block
    in final scheduled order, track the last-loaded weights AP, and for a
    repeat load merge its deps into the following matmul and delete it.  Only
    PE-engine instructions can clobber array state, so interleaved
    DVE/ACT/DMA instructions don't reset tracking.  Valid while weight
    source tiles are written exactly once (true here: xo is DMA'd once
    before any matmul).
    """
    removed = 0
    for bb in nc.m.functions[0].blocks:
        insts = bb.instructions
        last_sig = None
        pending = None
        to_remove = []
        renames = {}
        for i in insts:
            tn = type(i).__name__
            if tn == "InstLdweights":
                sig = (str(i.ins[0]), str(i.perf_mode), str(i.tile_position))
                if sig == last_sig:
                    pending = i
                else:
                    last_sig = sig
                    pending = None
            elif tn == "InstMatmult":
                if i.is_transpose:
                    last_sig = None
                    pending = None
                elif pending is not None:
                    i.merge_dependencies_from(pending)
                    to_remove.append((pending, i.name))
                    pending = None
            elif getattr(i, "engine", None) == _PE and tn not in (
                "InstEventSemaphore",
                "InstDrain",
            ):
                last_sig = None
                pending = None
        for (r, mmname) in to_remove:
            renames[r.name] = mmname
            insts.remove(r)
            removed += 1
        if renames:
            for i in insts:
                d = i.descendants
                if d is None:
                    continue
                for old, new in renames.items():
                    if old in d:
                        d.discard(old)
                        d.add(new)
    return removed


# ---------------------------------------------------------------- host prep
def _prep_shared(J, h_pos):
    """tril-mask, scale, transpose J into the packed (128, TOTW) fp8 rhs."""
    J = np.asarray(J, np.float32)
    h = np.asarray(h_pos, np.float32)
    mask = np.tril(np.ones((L, L), np.float32), k=-1)
    out = np.zeros((128, TOTW), FP8)
    for kt in range(KT):
        j0 = JPK * kt
        jw = min(JPK, L - j0)
        blk = J[:, j0 : j0 + jw] * mask[:, j0 : j0 + jw, None, None]  # (i,j',a,b)
        t = blk.transpose(1, 3, 0, 2).reshape(jw * Q, LQ)  # rows=(j',b), cols=(i,a)
        tl = np.zeros((128, LQ), np.float32)
        tl[: jw * Q] = t
        if kt == 0:
            tl[126] = h.reshape(LQ)
        tl *= SCALE
        out[:, JOFS[kt] : JOFS[kt] + REALW[kt]] = tl[:, 504 * BMIN[kt] :].astype(
            FP8
        )
    return out


def _prep_core(Xs, Ws):
    """Per-core one-hot (both orientations) + weight tile."""
    Xs = np.asarray(Xs)
    jj = np.arange(L)
    # K-oriented one-hot: (KT, 128, MC); row = 21*(j%6)+b, col = m
    xoht = np.zeros((KT, 128, MC), np.float32)
    rows = Q * (jj % JPK)[None, :] + Xs  # (MC, L)
    kts = (jj // JPK)[None, :].repeat(MC, 0)  # (MC, L)
    mm = np.arange(MC)[:, None].repeat(L, 1)
    xoht[kts.ravel(), rows.ravel(), mm.ravel()] = 1.0
    xoht[0, 126, :] = 1.0  # bias row pairs with h row in J
    xoht = np.ascontiguousarray(
        xoht.transpose(1, 0, 2).reshape(128, KT * MC)
    )
    # m-oriented one-hot, host-built and streamed per i-block-pair: the
    # on-device build (is_equal with broadcast APs) cost 637ns/i-block of
    # DVE, and the DVE epilogue paces the whole kernel.  bf16 because fp8
    # on the DVE hard-crashes the device.
    ohm = np.ascontiguousarray(
        (Xs.reshape(MT, 128, L)[..., None] == np.arange(Q))
        .transpose(1, 0, 2, 3)
        .reshape(128, MT * LQ)
    ).astype(BF16)
    wt = np.ascontiguousarray(np.asarray(Ws, np.float32).reshape(MT, 128).T)
    return xoht.astype(FP8), ohm, wt


# ---------------------------------------------------------------- device code
def _build_graph(opts=None):
    o = {
        # 3 waves of <=4 psum banks: wave k of m-tile t reuses banks freed
        # two waves earlier, so matmuls never wait on the exp-paced release
        # trickle (which made the scheduler dive across lhs groups and
        # reload weights per matmul)
        "waves": ((0, 4), (4, 8), (8, NIB)),
        "bufs": (6, 4),               # epool, opool depths
    }
    o.update(opts or {})
    WAVES = list(o["waves"])
    NSEG = len(WAVES)
    EB, OB = o["bufs"]
    nc = bacc.Bacc(
        "TRN2", target_bir_lowering=False, debug=False, num_devices=NCORES
    )
    jd = nc.dram_tensor("jrs", [128, TOTW], _DT.float8e4, kind="ExternalInput")
    xd = nc.dram_tensor("xoht", [128, KT * MC], _DT.float8e4, kind="ExternalInput")
    ohd = nc.dram_tensor("ohm", [128, MT * LQ], _DT.bfloat16, kind="ExternalInput")
    wd = nc.dram_tensor("wt", [128, MT], _DT.float32, kind="ExternalInput")
    outd = nc.dram_tensor("out", [128, 1], _DT.float32, kind="ExternalOutput")

    f32, fp8, bf16 = _DT.float32, _DT.float8e4, _DT.bfloat16
    OHW = 2 * IB * Q              # 1008 cols = one i-block pair per oh DMA
    NOH = (LQ + OHW - 1) // OHW   # 6 oh tiles per m-tile (5x1008 + 336)

    with tile.TileContext(nc) as tc:
        with (
            tc.tile_pool(name="jres", bufs=1) as jpool,
            tc.tile_pool(name="xres", bufs=1) as xpool,
            tc.tile_pool(name="consts", bufs=1) as cpool,
            tc.tile_pool(name="psum", bufs=8, space="PSUM") as ppool,
            tc.tile_pool(name="exps", bufs=EB) as epool,
            tc.tile_pool(name="ohms", bufs=OB) as opool,
            tc.tile_pool(name="small", bufs=4) as spool,
        ):
            jt = jpool.tile([128, TOTW], fp8)
            xo = xpool.tile([128, KT * MC], fp8)

            def jt_dma(a, b):
                nc.sync.dma_start(
                    jt[:, JOFS[a] : JOFS[b]], jd[:, JOFS[a] : JOFS[b]]
                )

            def xo_dma(a, b):
                nc.sync.dma_start(
                    xo[:, a * MC : b * MC], xd[:, a * MC : b * MC]
                )

            # Every dma_start costs ~700ns of serialized SP-queue issue
            # time; 112 of them meant the one-hot fetches (emitted inside
            # the waves, hence queued behind the preload bulk) did not
            # even ISSUE until ~80us -- the first multiply ran at 87us,
            # e-tiles could not recycle, and the exp->bank-release path
            # stalled the PE through the whole ramp.  So: fine-grained
            # chunks only for the first K-tiles the ramp needs, merged
            # chunks for the rest (44 issues total), and t0's one-hot is
            # prefetched right after the first J chunks (see oh_pre).
            jt_dma(0, 1)
            jt_dma(1, 2)
            xo_dma(0, 11)
            wt = cpool.tile([128, MT], f32)
            nc.sync.dma_start(wt[:], wd[:])

            # combined per-(m-tile, i-block) accumulator: 24 softmax-denominator
            # segments then 24 gold-exp(z) segments per block (bf16; the gold
            # sums are exact -- one nonzero per 21-seg).  Unused pad columns of
            # the short last block are preset to 1.0 so the per-m-tile Ln
            # accumulation sees ln(1)=0 there.
            CBW = 2 * IB * NIB  # 528 accumulator columns per m-tile
            cbig = cpool.tile([128, MT * CBW], bf16)
            nc.gpsimd.memset(cbig[:], 1.0)
            cb_pitch = cbig[:].ap[0][0]

            def fetch_oh(t, ib_lo, ib_hi):
                # one DMA per (m-tile, seg): 3-4KB per-partition lines
                c0 = t * LQ + ib_lo * IB * Q
                w = (min(ib_hi * IB, L) - ib_lo * IB) * Q
                oht = opool.tile(
                    [128, 4 * IB * Q], bf16, tag="oh", name=f"oh_{t}_{ib_lo}"
                )
                nc.sync.dma_start(oht[:, :w], ohd[:, c0 : c0 + w])
                return oht

            def epilogue(t, ib, ib_lo, ps, oht):
                w = IB_N[ib]
                nI = w // Q
                # exp on ACT is the psum bank's ONLY reader: banks release at
                # ACT pace (~760ns) instead of the 2.5us DVE chain
                e = epool.tile([128, 1008], bf16, tag="exp")
                nc.scalar.activation(
                    e[:, :w], ps[:, :w], mybir.ActivationFunctionType.Exp,
                    scale=1.0 / SCALE,
                )
                # gold extraction in exp domain: e*onehot has exactly one
                # nonzero per i-segment, so its 21-wide segment-reduce is the
                # gold exp(z) exactly; Ln+accum per m-tile recovers
                # sum_i z_gold.  All-bf16 SBUF operands engage DVE 2x_1P for
                # the multiply; writing into the back half of the same tile
                # lets ONE segment-reduce cover denominators and golds.
                ohs = oht[:, (ib - ib_lo) * 504 : (ib - ib_lo) * 504 + w]
                nc.vector.tensor_tensor(
                    out=e[:, w : 2 * w], in0=e[:, :w], in1=ohs,
                    op=mybir.AluOpType.mult,
                )
                e_pitch = e[:].ap[0][0]
                with nc.allow_low_precision(
                    reason="21-term bf16 sums; gold sums are exact"
                ):
                    nc.vector.reduce_sum(
                        bass.AP(
                            cbig.tensor,
                            int(cbig.offset + t * CBW + 2 * IB * ib),
                            [[int(cb_pitch), 128], [IB, 2], [1, nI]],
                        ),
                        bass.AP(
                            e.tensor, int(e.offset),
                            [[int(e_pitch), 128], [Q, 2 * nI], [1, Q]],
                        ),
                        axis=mybir.AxisListType.X,
                    )

            jt_pitch = jt[:].ap[0][0]
            xo_pitch = xo[:].ap[0][0]

            # Chain matmuls that share a stationary operand with NO_SYNC
            # (same-engine-order, no semaphore) edges so same-lhs groups stay
            # adjacent for _prune_ldweights.
            _NOSYNC = mybir._bass_rust.DependencyInfo.NO_SYNC_ONLY
            _prev_mm = [None]

            def chained_matmul(*args, **kwargs):
                r = nc.tensor.matmul(*args, **kwargs)
                if _prev_mm[0] is not None:
                    r.ins.add_dependency(_prev_mm[0], _NOSYNC)
                _prev_mm[0] = r.ins.name
                return r

            def new_group():
                # global chain: with 4+4+3-bank segs a bank's reuse trails
                # its release by ~2 segs of matmul work, so pinning the full
                # emission order costs no PE stalls (unlike the old 8-bank
                # waves) and keeps every same-lhs group contiguous
                pass

            def jt3(c0, step1, w):
                return bass.AP(
                    jt.tensor, int(jt.offset + c0),
                    [[int(jt_pitch), 128], [int(step1), 2], [1, int(w)]],
                )

            def xo3(p, t):
                return bass.AP(
                    xo.tensor, int(xo.offset + 2 * p * MC + t * 128),
                    [[int(xo_pitch), 128], [MC, 2], [1, 128]],
                )

            def run_wave(t, ib_lo, ib_hi):
                oht = oh_pre.pop((t, ib_lo), None)
                if oht is None:
                    oht = fetch_oh(t, ib_lo, ib_hi)
                psums = {}
                for ib in range(ib_lo, ib_hi):
                    psums[ib] = ppool.tile(
                        [128, 504], f32, tag="ps", name=f"ps_{t}_{ib}"
                    )
                pair_hi = (max(LASTKT[ib] for ib in range(ib_lo, ib_hi)) + 1) // 2
                for p in range(pair_hi):
                    kt = 2 * p
                    lhs = xo3(p, t)
                    new_group()
                    for ib in range(max(ib_lo, BMIN[kt]), ib_hi):
                        if kt > LASTKT[ib]:
                            continue
                        w = IB_N[ib]
                        c0 = int(JOFS[kt]) + 504 * (ib - BMIN[kt])
                        # in the pair's first i-block, columns i <= 12p are
                        # fully masked -- skip streaming them.  Pair 0 must
                        # stay full width: it carries the h bias row (valid
                        # for every i) and the start=True PSUM clear.
                        off = 0
                        if p > 0 and ib == BMIN[kt]:
                            off = max(0, (JPK * kt + 1 - IB * ib)) * Q
                        chained_matmul(
                            psums[ib][:, off:w],
                            lhs,
                            jt3(c0 + off, JW[kt], w - off),
                            start=(p == 0),
                            stop=(kt + 1 == LASTKT[ib]),
                            perf_mode=mybir.MatmulPerfMode.DoubleRow,
                        )
                # leftover odd K-tile 42 (j 252..255) -- plain matmul, ib10 only
                if ib_hi == NIB:
                    kt = KT - 1
                    ib = NIB - 1
                    w = IB_N[ib]
                    c0 = int(JOFS[kt]) + 504 * (ib - BMIN[kt])
                    off = max(0, (JPK * kt + 1 - IB * ib)) * Q
                    new_group()
                    chained_matmul(
                        psums[ib][:, off:w],
                        xo[:, kt * MC + t * 128 : kt * MC + (t + 1) * 128],
                        jt[:, c0 + off : c0 + w],
                        start=False,
                        stop=True,
                    )
                for ib in range(ib_lo, ib_hi):
                    epilogue(t, ib, ib_lo, psums[ib], oht)

            zcols = cpool.tile([128, MT], f32)
            gcols = cpool.tile([128, MT], f32)
            lns = cpool.tile([128, IB * NIB], f32)
            lns_pitch = None

            def run_ln(t):
                # ln of this m-tile's softmax denominators / gold exp(z)
                # values incl. the ln(1)=0 pads; accum_out gives the
                # per-partition i-sums directly
                for half, cols in ((0, zcols), (1, gcols)):
                    nc.scalar.activation(
                        lns[:],
                        bass.AP(
                            cbig.tensor,
                            int(cbig.offset + t * CBW + half * IB),
                            [[int(cb_pitch), 128], [2 * IB, NIB], [1, IB]],
                        ),
                        mybir.ActivationFunctionType.Ln,
                        accum_out=cols[:, t : t + 1],
                    )

            # sequential (t, seg) emission: with 4+4+3-bank segs on the
            # 8-buffer psum pool, a bank's reacquisition trails its
            # exp-release by ~2 segs of matmul work, so the PE never waits
            # on the release trickle (8-bank waves stalled at every wave
            # start, making the scheduler dive across lhs groups and reload
            # weights per matmul).  A seg-major sweep was tried and is
            # worse: it drains all shallow-J work first and then sits 27us
            # against the HBM wall waiting for deep J columns.  Lns batch
            # in two groups (Exp and Ln do NOT share an ACT table set; each
            # switch costs a 1.28us reload that stalls the release-pacing
            # ACT queue).
            # t0's one-hot prefetch, then the remaining preload bulk
            oh_pre = {}
            for s in range(NSEG):
                oh_pre[(0, WAVES[s][0])] = fetch_oh(0, *WAVES[s])
            for kt in range(2, 12):
                jt_dma(kt, kt + 1)
            xo_dma(11, 22)
            for kt in range(12, 16):
                jt_dma(kt, kt + 1)
            jt_dma(16, 18)
            jt_dma(18, 21)
            xo_dma(22, 33)
            jt_dma(21, 23)
            jt_dma(23, 26)
            jt_dma(26, 28)
            jt_dma(28, 31)
            xo_dma(33, KT)
            jt_dma(31, 37)
            jt_dma(37, KT)

            # (a 2-m-tile seg2 lag was tried for extra deep-J DMA lead and
            # measured ~11us WORSE at matched clock state -- the longer
            # shallow prefix re-trips the HAM clock gate more)
            for t in range(MT):
                for s in range(NSEG):
                    run_wave(t, *WAVES[s])
                if t == MT - 2:
                    for u in range(MT - 1):
                        run_ln(u)
            run_ln(MT - 1)

            # final combine: per-partition partial of the data NLL
            # (regJ / regH are added on the host in float64)
            dm = spool.tile([128, MT], f32, tag="dm")
            nc.vector.tensor_tensor(
                out=dm[:], in0=zcols[:], in1=gcols[:],
                op=mybir.AluOpType.subtract,
            )
            wprod = spool.tile([128, MT], f32, tag="wprod")
            nc.vector.tensor_tensor(
                out=wprod[:], in0=dm[:], in1=wt[:], op=mybir.AluOpType.mult
            )
            nll = spool.tile([128, 1], f32, tag="nll")
            nc.vector.reduce_sum(nll[:], wprod[:], axis=mybir.AxisListType.X)
            nc.sync.dma_start(outd[:], nll[:])

    _prune_ldweights(nc)
    nc.compile()
    return nc


_GRAPH = None


def _graph():
    global _GRAPH
    if _GRAPH is None:
        _GRAPH = _build_graph()
    return _GRAPH


# ------------------------------------------------------- persistent runner
# Mirrors concourse.bass2jax.run_bass_via_pjrt but caches the jitted
# shard_map executable so repeated calls don't re-trace/re-compile.
class _Runner:
    def __init__(self, nc):
        import jax
        from jax.sharding import Mesh, PartitionSpec
        from jax.experimental.shard_map import shard_map
        import concourse.mybir as mybir
        from concourse import bass2jax

        bass2jax.install_neuronx_cc_hook()
        partition_name = (
            nc.partition_id_tensor.name if nc.partition_id_tensor else None
        )
        in_names, out_names, out_avals, zero_outs = [], [], [], []
        for alloc in nc.m.functions[0].allocations:
            if not isinstance(alloc, mybir.MemoryLocationSet):
                continue
            name = alloc.memorylocations[0].name
            if alloc.kind == "ExternalInput":
                if name != partition_name:
                    in_names.append(name)
            elif alloc.kind == "ExternalOutput":
                shape = tuple(alloc.tensor_shape)
                dtype = mybir.dt.np(alloc.dtype)
                out_names.append(name)
                out_avals.append(jax.core.ShapedArray(shape, dtype))
                zero_outs.append(np.zeros(shape, dtype))
        n_params = len(in_names)
        all_names = in_names + out_names
        if partition_name is not None:
            all_names = all_names + [partition_name]

        def _body(*args):
            operands = list(args)
            if partition_name is not None:
                operands.append(bass2jax.partition_id_tensor())
            outs = bass2jax._bass_exec_p.bind(
                *operands,
                out_avals=tuple(out_avals),
                in_names=tuple(all_names),
                out_names=tuple(out_names),
                lowering_input_output_aliases=(),
                sim_require_finite=True,
                sim_require_nnan=True,
                nc=nc,
            )
            return tuple(outs)

        devices = jax.devices()[:NCORES]
        mesh = Mesh(np.asarray(devices), ("core",))
        self.mesh = mesh
        nin = n_params + len(out_names)
        self._jit = jax.jit(
            shard_map(
                _body,
                mesh=mesh,
                in_specs=(PartitionSpec("core"),) * nin,
                out_specs=(PartitionSpec("core"),) * len(out_names),
                check_rep=False,
            ),
            keep_unused=True,
        )
        self.in_names = in_names
        self.out_names = out_names
        self.out_avals = out_avals
        self.zero_outs = zero_outs
        self._jax = jax

    def put_inputs(self, in_maps, device_resident=True):
        """Concatenate per-core inputs and return the arg list."""
        concat = [
            np.concatenate(
                [np.asarray(in_maps[c][n]) for c in range(NCORES)], axis=0
            )
            for n in self.in_names
        ]
        zeros = [
            np.zeros((NCORES * z.shape[0], *z.shape[1:]), z.dtype)
            for z in self.zero_outs
        ]
        args = concat + zeros
        if device_resident:
            from jax.sharding import NamedSharding, PartitionSpec

            sh = NamedSharding(self.mesh, PartitionSpec("core"))
            args = [self._jax.device_put(a, sh) for a in args]
            self._jax.block_until_ready(args)
        return args

    def run(self, args):
        outs = self._jit(*args)
        self._jax.block_until_ready(outs)
        return {
            n: np.asarray(outs[i]).reshape(NCORES, *self.out_avals[i].shape)
            for i, n in enumerate(self.out_names)
        }


_RUNNER = None


def _runner():
    global _RUNNER
    if _RUNNER is None:
        _RUNNER = _Runner(_graph())
    return _RUNNER


def _make_in_maps(X_idx, W, h_pos, J):
    X_idx = np.asarray(X_idx)
    W = np.asarray(W, np.float32)
    jrs = _prep_shared(J, h_pos)
    in_maps = []
    for c in range(NCORES):
        xoht, ohm, wt = _prep_core(
            X_idx[c * MC : (c + 1) * MC], W[c * MC : (c + 1) * MC]
        )
        in_maps.append({"jrs": jrs, "xoht": xoht, "ohm": ohm, "wt": wt})
    return in_maps


def _reg_terms(h_pos, J):
    """Exact float64 regularizers, done on the host (the device computes only
    the data NLL)."""
    J = np.asarray(J, np.float64)
    mask = np.tril(np.ones((L, L)), k=-1)
    regJ = float(np.einsum("ijab,ijab,ij->", J, J, mask))
    h = np.asarray(h_pos, np.float64)
    regH = float((h * h).sum())
    return LAMBDA_J * regJ + LAMBDA_H * regH


# ---------------------------------------------------------------- entry point
def kernel(X_idx, W, h_pos, J):
    in_maps = _make_in_maps(X_idx, W, h_pos, J)
    try:
        r = _runner()
        out = r.run(r.put_inputs(in_maps))["out"]
    except Exception:
        # stock execution path (slower dispatch, same NEFF)
        res = run_bass_kernel_spmd(
            _graph(), in_maps, core_ids=list(range(NCORES))
        )
        out = np.stack([np.asarray(res.results[c]["out"]) for c in range(NCORES)])
    return np.float32(
        np.asarray(out, np.float64).sum() + _reg_terms(h_pos, J)
    )


def bench(X_idx, W, h_pos, J, reps=20):
    """Return (loss, mean_exec_seconds) amortized over reps (incl. RPC)."""
    import time

    r = _runner()
    args = r.put_inputs(_make_in_maps(X_idx, W, h_pos, J))
    out = r.run(args)  # warm-up / compile
    t0 = time.time()
    for _ in range(reps):
        out = r.run(args)
    dt = (time.time() - t0) / reps
    loss = np.asarray(out["out"], np.float64).sum() + _reg_terms(h_pos, J)
    return np.float32(loss), dt



# revision 40
# speedup vs baseline: 1.0378x; 1.0378x over previous
"""ArDCA pseudo-likelihood loss on 8 Trainium2 NeuronCores.

Math (reference): for samples X (M,L) over alphabet Q with weights W,
    pair[m,i,a] = sum_{j<i} J[i,j,a,X[m,j]]
    logits = h_pos + pair ;  loss = -sum_{m,i} W[m]*log_softmax(logits)[gold]
                              + lam_h*|h|^2 + lam_j*|tril(J)|^2

Strategy: data-parallel over M (1024 samples/core).  The one-hot einsum is a
dense TensorEngine matmul: out[m, (i,a)] += onehotT[(j,b), m].T @ J[(j,b),(i,a)]
with K = L*Q = 5376 contraction packed 6 j-positions per 128-row K-tile and the
strict lower-triangle (j<i) skipped at tile granularity.  J (tril-masked,
scaled by 64, fp8-e4m3, column-packed) stays SBUF-resident and streams through
the PE as fp8 DoubleRow pairs (two K-tiles per matmul, 2 weights/cell); h_pos
rides in as a bias row of K-tile 0 against an all-ones row of the one-hot.
Each m-tile runs as three 4+4+3-psum-bank segs so a bank's reacquisition
trails its release by ~2 segs of matmul work.  Epilogue per (m-tile,
i-block): exp on ScalarE (bf16 out, scale=1/64 folded in, sole psum reader
so banks release at ACT pace), gold extraction in exp domain (e*onehot with
the host-built, DMA-streamed one-hot has one nonzero per 21-seg), one merged
segment-reduce on the DVE covering denominators and golds, then two
Ln+accum per m-tile (batched at the end -- Exp and Ln don't share an ACT
table set).  regJ/regH are float64 on the host.  Each core emits a (128,1)
partial NLL column; the host sums them and adds the regularizers.

Two post-scheduling BIR passes matter: a global NO_SYNC chain pins the PE
stream to emission order (the tile scheduler otherwise orders matmuls
i-block-major, one weight reload per matmul), and _prune_ldweights dedups
the 256-column DoubleRow LDWEIGHTS within each same-lhs group (1056 -> 368
weight loads, ~60us of TensorE time).

Known landmines on this stack (found the hard way): tensor_tensor_reduce and
any fp8 operand on the VectorEngine hard-crash the device; engine APs must
start at 32-aligned partitions; interleaving Exp/Ln per-tile thrashes ACT
table sets (~1.3us per reload); walrus --enable-ldw-opt crashes codegen on
DoubleRow; post-hoc PE stream reordering (see _repack_pe_order, unused)
deadlocks the device -- order must be imposed via dependencies so the
scheduler's bank/slot bookkeeping stays consistent.
"""

import os
import sys

import numpy as np
import ml_dtypes

try:
    import concourse.bass as bass  # noqa: F401
except ImportError:  # pragma: no cover
    sys.path.insert(0, "/opt/trn_rl_repo")

import concourse.bass as bass
import concourse.mybir as mybir
import concourse.tile as tile
from concourse import bacc
from concourse import bass_utils as _bu
from concourse.bass_utils import run_bass_kernel_spmd



# ---------------------------------------------------------------- constants
M, L, Q = 8192, 256, 21
LAMBDA_H = 1e-06
LAMBDA_J = 0.0001

NCORES = 8
MC = M // NCORES        # 1024 samples per core
MT = MC // 128          # 8 m-tiles per core
LQ = L * Q              # 5376

JPK = 6                 # j-positions per K-tile (6*21=126 <= 128)
KT = (L + JPK - 1) // JPK   # 43 K-tiles
IB = 24                 # i-positions per i-block (24*21=504 <= 512 psum bank)
NIB = (L + IB - 1) // IB    # 11 i-blocks (10 of 24 + 1 of 16)
IB_N = [min(IB, L - IB * b) * Q for b in range(NIB)]  # 504 .. 336
SCALE = 64.0            # fp8 pre-scale on J / h

FP8 = ml_dtypes.float8_e4m3
BF16 = ml_dtypes.bfloat16

# first i-block each K-tile contributes to: need some i in block with i > 6*kt
BMIN = [(JPK * kt + 1) // IB for kt in range(KT)]
assert all(BMIN[kt] == kt // 4 for kt in range(KT))
# last K-tile contributing to i-block b:  j <= i_max-1 = min(IB*(b+1),L)-2
LASTKT = [min(KT - 1, (IB * (b + 1) - 2) // JPK) for b in range(NIB)]
# packed J column widths / offsets (per partition, in elements).  Widths are
# padded to 16 so the DoubleRow middle-dim step (= JW of the even K-tile of a
# pair) satisfies the ISA's step%16 requirement.
REALW = [LQ - 504 * BMIN[kt] for kt in range(KT)]
JW = [(w + 15) // 16 * 16 for w in REALW]
JOFS = np.concatenate([[0], np.cumsum(JW)]).astype(int)
TOTW = int(JOFS[-1])
NPAIR = KT // 2  # 21 DoubleRow pairs; K-tile 42 runs as a plain matmul

WAVES = [(0, 8), (8, NIB)]  # i-block ranges sharing the 8 psum banks

_DT = mybir.dt

# PE engine type for the LDWEIGHTS prune below (EngineType.PE)
_PE = mybir.EngineType.PE


def _repack_pe_order(nc):
    """Re-serialize each block with the PE stream in emission order.

    The tile scheduler orders matmuls by simulated readiness, which in
    steady state degenerates to i-block-major order across lhs groups --
    forcing a 256-column LDWEIGHTS reload before nearly every matmul.  The
    emission order (pair-major within each 4-bank seg) keeps same-lhs
    matmuls adjacent and is provably safe: per-bank accumulation chains
    stay ascending, and a seg's start=True matmuls only wait on exps of
    two segs earlier.  Rebuild the block as a dependency-respecting
    topological merge of per-engine streams, with the PE stream replaced
    by (LDWEIGHTS, MATMUL) units sorted by matmul emission id.  Runs
    before nc.compile(), so semaphores are generated from the new order.
    """
    for bb in nc.m.functions[0].blocks:
        insts = list(bb.instructions)
        # split into segments at barrier instructions (branches & friends)
        segments = []
        cur = []
        barriers = []
        for i in insts:
            tn = type(i).__name__
            if tn in ("InstLdweights", "InstMatmult") or getattr(
                i, "engine", None
            ) is not None and tn not in (
                "InstUnconditionalBranch",
                "InstCompareAndBranch",
                "InstBranchHint",
            ):
                cur.append(i)
            else:
                segments.append(cur)
                barriers.append(i)
                cur = []
        segments.append(cur)

        out = []
        for seg_idx, seg in enumerate(segments):
            n_mm = sum(1 for i in seg if type(i).__name__ == "InstMatmult")
            if n_mm < 2:
                out.extend(seg)
            else:
                out.extend(_topo_merge_pe_emission(seg))
            if seg_idx < len(barriers):
                out.append(barriers[seg_idx])
        assert len(out) == len(insts)
        bb.instructions.clear()
        bb.instructions.extend(out)


def _topo_merge_pe_emission(seg):
    from collections import defaultdict

    streams = defaultdict(list)   # engine -> instruction stream
    pe_units = []
    pending_ldw = None
    for i in seg:
        tn = type(i).__name__
        if tn == "InstLdweights":
            assert pending_ldw is None
            pending_ldw = i
        elif tn == "InstMatmult":
            unit = ([pending_ldw] if pending_ldw is not None else []) + [i]
            pending_ldw = None
            pe_units.append((int(i.name.split("-")[1]), unit))
        else:
            streams[i.engine].append([i])
    assert pending_ldw is None
    pe_units.sort(key=lambda u: u[0])
    streams[_PE] = [u for _, u in pe_units]

    local = {
        i.name
        for units in streams.values()
        for unit in units
        for i in unit
    }
    emitted = set()
    heads = {e: 0 for e in streams}
    result = []
    total = sum(len(u) for us in streams.values() for u in us)
    while len(result) < total:
        progress = False
        for e, units in streams.items():
            while heads[e] < len(units):
                unit = units[heads[e]]
                deps = set()
                names = {i.name for i in unit}
                for i in unit:
                    deps |= set(i.sync_dependency_set_copy())
                    deps |= set(i.nosync_dependency_set_copy())
                deps -= names
                if (deps & local) - emitted:
                    break
                result.extend(unit)
                emitted |= names
                heads[e] += 1
                progress = True
        if not progress:
            raise RuntimeError("topo merge stalled (dependency cycle?)")
    return result


def _prune_ldweights(nc):
    """Remove InstLdweights that reload the identical stationary operand.

    The tile legalizer splits every fp8 matmul into LDWEIGHTS+MATMUL; with
    DoubleRow the 256-column weight load (~200ns) costs ~2x the ~105ns matmul
    stream, and 10 of every 11 loads here re-load the same one-hot tile (the
    inner i-block loop shares lhs).  Walrus's own dedup (--enable-ldw-opt)
    crashes codegen on DoubleRow, so dedup at the BIR level: scan each block
    in final scheduled order, track the last-loaded weights AP, and for a
    repeat load merge its deps into the following matmul and delete it.  Only
    PE-engine instructions can clobber array state, so interleaved
    DVE/ACT/DMA instructions don't reset tracking.  Valid while weight
    source tiles are written exactly once (true here: xo is DMA'd once
    before any matmul).
    """
    removed = 0
    for bb in nc.m.functions[0].blocks:
        insts = bb.instructions
        last_sig = None
        pending = None
        to_remove = []
        renames = {}
        for i in insts:
            tn = type(i).__name__
            if tn == "InstLdweights":
                sig = (str(i.ins[0]), str(i.perf_mode), str(i.tile_position))
                if sig == last_sig:
                    pending = i
                else:
                    last_sig = sig
                    pending = None
            elif tn == "InstMatmult":
                if i.is_transpose:
                    last_sig = None
                    pending = None
                elif pending is not None:
                    i.merge_dependencies_from(pending)
                    to_remove.append((pending, i.name))
                    pending = None
            elif getattr(i, "engine", None) == _PE and tn not in (
                "InstEventSemaphore",
                "InstDrain",
            ):
                last_sig = None
                pending = None
        for (r, mmname) in to_remove:
            renames[r.name] = mmname
            insts.remove(r)
            removed += 1
        if renames:
            for i in insts:
                d = i.descendants
                if d is None:
                    continue
                for old, new in renames.items():
                    if old in d:
                        d.discard(old)
                        d.add(new)
    return removed


# ---------------------------------------------------------------- host prep
def _prep_shared(J, h_pos):
    """tril-mask, scale, transpose J into the packed (128, TOTW) fp8 rhs."""
    J = np.asarray(J, np.float32)
    h = np.asarray(h_pos, np.float32)
    mask = np.tril(np.ones((L, L), np.float32), k=-1)
    out = np.zeros((128, TOTW), FP8)
    for kt in range(KT):
        j0 = JPK * kt
        jw = min(JPK, L - j0)
        blk = J[:, j0 : j0 + jw] * mask[:, j0 : j0 + jw, None, None]  # (i,j',a,b)
        t = blk.transpose(1, 3, 0, 2).reshape(jw * Q, LQ)  # rows=(j',b), cols=(i,a)
        tl = np.zeros((128, LQ), np.float32)
        tl[: jw * Q] = t
        if kt == 0:
            tl[126] = h.reshape(LQ)
        tl *= SCALE
        out[:, JOFS[kt] : JOFS[kt] + REALW[kt]] = tl[:, 504 * BMIN[kt] :].astype(
            FP8
        )
    return out


def _prep_core(Xs, Ws):
    """Per-core one-hot (both orientations) + weight tile."""
    Xs = np.asarray(Xs)
    jj = np.arange(L)
    # K-oriented one-hot: (KT, 128, MC); row = 21*(j%6)+b, col = m
    xoht = np.zeros((KT, 128, MC), np.float32)
    rows = Q * (jj % JPK)[None, :] + Xs  # (MC, L)
    kts = (jj // JPK)[None, :].repeat(MC, 0)  # (MC, L)
    mm = np.arange(MC)[:, None].repeat(L, 1)
    xoht[kts.ravel(), rows.ravel(), mm.ravel()] = 1.0
    xoht[0, 126, :] = 1.0  # bias row pairs with h row in J
    xoht = np.ascontiguousarray(
        xoht.transpose(1, 0, 2).reshape(128, KT * MC)
    )
    # m-oriented one-hot, host-built and streamed per i-block-pair: the
    # on-device build (is_equal with broadcast APs) cost 637ns/i-block of
    # DVE, and the DVE epilogue paces the whole kernel.  bf16 because fp8
    # on the DVE hard-crashes the device.
    ohm = np.ascontiguousarray(
        (Xs.reshape(MT, 128, L)[..., None] == np.arange(Q))
        .transpose(1, 0, 2, 3)
        .reshape(128, MT * LQ)
    ).astype(BF16)
    wt = np.ascontiguousarray(np.asarray(Ws, np.float32).reshape(MT, 128).T)
    return xoht.astype(FP8), ohm, wt


# ---------------------------------------------------------------- device code
def _build_graph(opts=None):
    o = {
        # 3 waves of <=4 psum banks: wave k of m-tile t reuses banks freed
        # two waves earlier, so matmuls never wait on the exp-paced release
        # trickle (which made the scheduler dive across lhs groups and
        # reload weights per matmul)
        "waves": ((0, 4), (4, 8), (8, NIB)),
        "bufs": (6, 4),               # epool, opool depths
    }
    o.update(opts or {})
    WAVES = list(o["waves"])
    NSEG = len(WAVES)
    EB, OB = o["bufs"]
    nc = bacc.Bacc(
        "TRN2", target_bir_lowering=False, debug=False, num_devices=NCORES
    )
    jd = nc.dram_tensor("jrs", [128, TOTW], _DT.float8e4, kind="ExternalInput")
    xd = nc.dram_tensor("xoht", [128, KT * MC], _DT.float8e4, kind="ExternalInput")
    ohd = nc.dram_tensor("ohm", [128, MT * LQ], _DT.bfloat16, kind="ExternalInput")
    wd = nc.dram_tensor("wt", [128, MT], _DT.float32, kind="ExternalInput")
    outd = nc.dram_tensor("out", [128, 1], _DT.float32, kind="ExternalOutput")

    f32, fp8, bf16 = _DT.float32, _DT.float8e4, _DT.bfloat16
    OHW = 2 * IB * Q              # 1008 cols = one i-block pair per oh DMA
    NOH = (LQ + OHW - 1) // OHW   # 6 oh tiles per m-tile (5x1008 + 336)

    with tile.TileContext(nc) as tc:
        with (
            tc.tile_pool(name="jres", bufs=1) as jpool,
            tc.tile_pool(name="xres", bufs=1) as xpool,
            tc.tile_pool(name="consts", bufs=1) as cpool,
            tc.tile_pool(name="psum", bufs=8, space="PSUM") as ppool,
            tc.tile_pool(name="exps", bufs=EB) as epool,
            tc.tile_pool(name="ohms", bufs=OB) as opool,
            tc.tile_pool(name="small", bufs=4) as spool,
        ):
            jt = jpool.tile([128, TOTW], fp8)
            xo = xpool.tile([128, KT * MC], fp8)

            def jt_dma(a, b):
                nc.sync.dma_start(
                    jt[:, JOFS[a] : JOFS[b]], jd[:, JOFS[a] : JOFS[b]]
                )

            def xo_dma(a, b):
                nc.sync.dma_start(
                    xo[:, a * MC : b * MC], xd[:, a * MC : b * MC]
                )

            # Every dma_start costs ~700ns of serialized SP-queue issue
            # time; 112 of them meant the one-hot fetches (emitted inside
            # the waves, hence queued behind the preload bulk) did not
            # even ISSUE until ~80us -- the first multiply ran at 87us,
            # e-tiles could not recycle, and the exp->bank-release path
            # stalled the PE through the whole ramp.  So: fine-grained
            # chunks only for the first K-tiles the ramp needs, merged
            # chunks for the rest (44 issues total), and t0's one-hot is
            # prefetched right after the first J chunks (see oh_pre).
            jt_dma(0, 1)
            jt_dma(1, 2)
            xo_dma(0, 11)
            wt = cpool.tile([128, MT], f32)
            nc.sync.dma_start(wt[:], wd[:])

            # combined per-(m-tile, i-block) accumulator: 24 softmax-denominator
            # segments then 24 gold-exp(z) segments per block (bf16; the gold
            # sums are exact -- one nonzero per 21-seg).  Unused pad columns of
            # the short last block are preset to 1.0 so the per-m-tile Ln
            # accumulation sees ln(1)=0 there.
            CBW = 2 * IB * NIB  # 528 accumulator columns per m-tile
            cbig = cpool.tile([128, MT * CBW], bf16)
            nc.gpsimd.memset(cbig[:], 1.0)
            cb_pitch = cbig[:].ap[0][0]

            def fetch_oh(t, ib_lo, ib_hi):
                # one DMA per (m-tile, seg): 3-4KB per-partition lines
                c0 = t * LQ + ib_lo * IB * Q
                w = (min(ib_hi * IB, L) - ib_lo * IB) * Q
                oht = opool.tile(
                    [128, 4 * IB * Q], bf16, tag="oh", name=f"oh_{t}_{ib_lo}"
                )
                nc.sync.dma_start(oht[:, :w], ohd[:, c0 : c0 + w])
                return oht

            def epilogue(t, ib, ib_lo, ps, oht):
                w = IB_N[ib]
                nI = w // Q
                # exp on ACT is the psum bank's ONLY reader: banks release at
                # ACT pace (~760ns) instead of the 2.5us DVE chain
                e = epool.tile([128, 1008], bf16, tag="exp")
                nc.scalar.activation(
                    e[:, :w], ps[:, :w], mybir.ActivationFunctionType.Exp,
                    scale=1.0 / SCALE,
                )
                # gold extraction in exp domain: e*onehot has exactly one
                # nonzero per i-segment, so its 21-wide segment-reduce is the
                # gold exp(z) exactly; Ln+accum per m-tile recovers
                # sum_i z_gold.  All-bf16 SBUF operands engage DVE 2x_1P for
                # the multiply; writing into the back half of the same tile
                # lets ONE segment-reduce cover denominators and golds.
                ohs = oht[:, (ib - ib_lo) * 504 : (ib - ib_lo) * 504 + w]
                nc.vector.tensor_tensor(
                    out=e[:, w : 2 * w], in0=e[:, :w], in1=ohs,
                    op=mybir.AluOpType.mult,
                )
                e_pitch = e[:].ap[0][0]
                with nc.allow_low_precision(
                    reason="21-term bf16 sums; gold sums are exact"
                ):
                    nc.vector.reduce_sum(
                        bass.AP(
                            cbig.tensor,
                            int(cbig.offset + t * CBW + 2 * IB * ib),
                            [[int(cb_pitch), 128], [IB, 2], [1, nI]],
                        ),
                        bass.AP(
                            e.tensor, int(e.offset),
                            [[int(e_pitch), 128], [Q, 2 * nI], [1, Q]],
                        ),
                        axis=mybir.AxisListType.X,
                    )

            jt_pitch = jt[:].ap[0][0]
            xo_pitch = xo[:].ap[0][0]

            # Chain matmuls that share a stationary operand with NO_SYNC
            # (same-engine-order, no semaphore) edges so same-lhs groups stay
            # adjacent for _prune_ldweights.
            _NOSYNC = mybir._bass_rust.DependencyInfo.NO_SYNC_ONLY
            _prev_mm = [None]

            def chained_matmul(*args, **kwargs):
                r = nc.tensor.matmul(*args, **kwargs)
                if _prev_mm[0] is not None:
                    r.ins.add_dependency(_prev_mm[0], _NOSYNC)
                _prev_mm[0] = r.ins.name
                return r

            def new_group():
                # global chain: with 4+4+3-bank segs a bank's reuse trails
                # its release by ~2 segs of matmul work, so pinning the full
                # emission order costs no PE stalls (unlike the old 8-bank
                # waves) and keeps every same-lhs group contiguous
                pass

            def jt3(c0, step1, w):
                return bass.AP(
                    jt.tensor, int(jt.offset + c0),
                    [[int(jt_pitch), 128], [int(step1), 2], [1, int(w)]],
                )

            def xo3(p, t):
                return bass.AP(
                    xo.tensor, int(xo.offset + 2 * p * MC + t * 128),
                    [[int(xo_pitch), 128], [MC, 2], [1, 128]],
                )

            def run_wave(t, ib_lo, ib_hi):
                oht = oh_pre.pop((t, ib_lo), None)
                if oht is None:
                    oht = fetch_oh(t, ib_lo, ib_hi)
                psums = {}
                for ib in range(ib_lo, ib_hi):
                    psums[ib] = ppool.tile(
                        [128, 504], f32, tag="ps", name=f"ps_{t}_{ib}"
                    )
                pair_hi = (max(LASTKT[ib] for ib in range(ib_lo, ib_hi)) + 1) // 2
                for p in range(pair_hi):
                    kt = 2 * p
                    lhs = xo3(p, t)
                    new_group()
                    for ib in range(max(ib_lo, BMIN[kt]), ib_hi):
                        if kt > LASTKT[ib]:
                            continue
                        w = IB_N[ib]
                        c0 = int(JOFS[kt]) + 504 * (ib - BMIN[kt])
                        # in the pair's first i-block, columns i <= 12p are
                        # fully masked -- skip streaming them.  Pair 0 must
                        # stay full width: it carries the h bias row (valid
                        # for every i) and the start=True PSUM clear.
                        off = 0
                        if p > 0 and ib == BMIN[kt]:
                            off = max(0, (JPK * kt + 1 - IB * ib)) * Q
                        chained_matmul(
                            psums[ib][:, off:w],
                            lhs,
                            jt3(c0 + off, JW[kt], w - off),
                            start=(p == 0),
                            stop=(kt + 1 == LASTKT[ib]),
                            perf_mode=mybir.MatmulPerfMode.DoubleRow,
                        )
                # leftover odd K-tile 42 (j 252..255) -- plain matmul, ib10 only
                if ib_hi == NIB:
                    kt = KT - 1
                    ib = NIB - 1
                    w = IB_N[ib]
                    c0 = int(JOFS[kt]) + 504 * (ib - BMIN[kt])
                    off = max(0, (JPK * kt + 1 - IB * ib)) * Q
                    new_group()
                    chained_matmul(
                        psums[ib][:, off:w],
                        xo[:, kt * MC + t * 128 : kt * MC + (t + 1) * 128],
                        jt[:, c0 + off : c0 + w],
                        start=False,
                        stop=True,
                    )
                for ib in range(ib_lo, ib_hi):
                    epilogue(t, ib, ib_lo, psums[ib], oht)

            zcols = cpool.tile([128, MT], f32)
            gcols = cpool.tile([128, MT], f32)
            lns = cpool.tile([128, IB * NIB], f32)
            lns_pitch = None

            def run_ln(t):
                # ln of this m-tile's softmax denominators / gold exp(z)
                # values incl. the ln(1)=0 pads; accum_out gives the
                # per-partition i-sums directly
                for half, cols in ((0, zcols), (1, gcols)):
                    nc.scalar.activation(
                        lns[:],
                        bass.AP(
                            cbig.tensor,
                            int(cbig.offset + t * CBW + half * IB),
                            [[int(cb_pitch), 128], [2 * IB, NIB], [1, IB]],
                        ),
                        mybir.ActivationFunctionType.Ln,
                        accum_out=cols[:, t : t + 1],
                    )

            # sequential (t, seg) emission: with 4+4+3-bank segs on the
            # 8-buffer psum pool, a bank's reacquisition trails its
            # exp-release by ~2 segs of matmul work, so the PE never waits
            # on the release trickle (8-bank waves stalled at every wave
            # start, making the scheduler dive across lhs groups and reload
            # weights per matmul).  A seg-major sweep was tried and is
            # worse: it drains all shallow-J work first and then sits 27us
            # against the HBM wall waiting for deep J columns.  Lns batch
            # in two groups (Exp and Ln do NOT share an ACT table set; each
            # switch costs a 1.28us reload that stalls the release-pacing
            # ACT queue).
            # t0's one-hot prefetch, then the remaining preload bulk
            oh_pre = {}
            for s in range(NSEG):
                oh_pre[(0, WAVES[s][0])] = fetch_oh(0, *WAVES[s])
            for kt in range(2, 12):
                jt_dma(kt, kt + 1)
            xo_dma(11, 22)
            for kt in range(12, 16):
                jt_dma(kt, kt + 1)
            jt_dma(16, 21)
            xo_dma(22, 33)
            jt_dma(21, 26)
            jt_dma(26, 31)
            xo_dma(33, KT)
            jt_dma(31, 37)
            jt_dma(37, KT)

            # (a 2-m-tile seg2 lag was tried for extra deep-J DMA lead and
            # measured ~11us WORSE at matched clock state -- the longer
            # shallow prefix re-trips the HAM clock gate more)
            for t in range(MT):
                for s in range(NSEG):
                    run_wave(t, *WAVES[s])
                if t == MT - 2:
                    for u in range(MT - 1):
                        run_ln(u)
            run_ln(MT - 1)

            # final combine: per-partition partial of the data NLL
            # (regJ / regH are added on the host in float64)
            dm = spool.tile([128, MT], f32, tag="dm")
            nc.vector.tensor_tensor(
                out=dm[:], in0=zcols[:], in1=gcols[:],
                op=mybir.AluOpType.subtract,
            )
            wprod = spool.tile([128, MT], f32, tag="wprod")
            nc.vector.tensor_tensor(
                out=wprod[:], in0=dm[:], in1=wt[:], op=mybir.AluOpType.mult
            )
            nll = spool.tile([128, 1], f32, tag="nll")
            nc.vector.reduce_sum(nll[:], wprod[:], axis=mybir.AxisListType.X)
            nc.sync.dma_start(outd[:], nll[:])

    _prune_ldweights(nc)
    nc.compile()
    return nc


_GRAPH = None


def _graph():
    global _GRAPH
    if _GRAPH is None:
        _GRAPH = _build_graph()
    return _GRAPH


# ------------------------------------------------------- persistent runner
# Mirrors concourse.bass2jax.run_bass_via_pjrt but caches the jitted
# shard_map executable so repeated calls don't re-trace/re-compile.
class _Runner:
    def __init__(self, nc):
        import jax
        from jax.sharding import Mesh, PartitionSpec
        from jax.experimental.shard_map import shard_map
        import concourse.mybir as mybir
        from concourse import bass2jax

        bass2jax.install_neuronx_cc_hook()
        partition_name = (
            nc.partition_id_tensor.name if nc.partition_id_tensor else None
        )
        in_names, out_names, out_avals, zero_outs = [], [], [], []
        for alloc in nc.m.functions[0].allocations:
            if not isinstance(alloc, mybir.MemoryLocationSet):
                continue
            name = alloc.memorylocations[0].name
            if alloc.kind == "ExternalInput":
                if name != partition_name:
                    in_names.append(name)
            elif alloc.kind == "ExternalOutput":
                shape = tuple(alloc.tensor_shape)
                dtype = mybir.dt.np(alloc.dtype)
                out_names.append(name)
                out_avals.append(jax.core.ShapedArray(shape, dtype))
                zero_outs.append(np.zeros(shape, dtype))
        n_params = len(in_names)
        all_names = in_names + out_names
        if partition_name is not None:
            all_names = all_names + [partition_name]

        def _body(*args):
            operands = list(args)
            if partition_name is not None:
                operands.append(bass2jax.partition_id_tensor())
            outs = bass2jax._bass_exec_p.bind(
                *operands,
                out_avals=tuple(out_avals),
                in_names=tuple(all_names),
                out_names=tuple(out_names),
                lowering_input_output_aliases=(),
                sim_require_finite=True,
                sim_require_nnan=True,
                nc=nc,
            )
            return tuple(outs)

        devices = jax.devices()[:NCORES]
        mesh = Mesh(np.asarray(devices), ("core",))
        self.mesh = mesh
        nin = n_params + len(out_names)
        self._jit = jax.jit(
            shard_map(
                _body,
                mesh=mesh,
                in_specs=(PartitionSpec("core"),) * nin,
                out_specs=(PartitionSpec("core"),) * len(out_names),
                check_rep=False,
            ),
            keep_unused=True,
        )
        self.in_names = in_names
        self.out_names = out_names
        self.out_avals = out_avals
        self.zero_outs = zero_outs
        self._jax = jax

    def put_inputs(self, in_maps, device_resident=True):
        """Concatenate per-core inputs and return the arg list."""
        concat = [
            np.concatenate(
                [np.asarray(in_maps[c][n]) for c in range(NCORES)], axis=0
            )
            for n in self.in_names
        ]
        zeros = [
            np.zeros((NCORES * z.shape[0], *z.shape[1:]), z.dtype)
            for z in self.zero_outs
        ]
        args = concat + zeros
        if device_resident:
            from jax.sharding import NamedSharding, PartitionSpec

            sh = NamedSharding(self.mesh, PartitionSpec("core"))
            args = [self._jax.device_put(a, sh) for a in args]
            self._jax.block_until_ready(args)
        return args

    def run(self, args):
        outs = self._jit(*args)
        self._jax.block_until_ready(outs)
        return {
            n: np.asarray(outs[i]).reshape(NCORES, *self.out_avals[i].shape)
            for i, n in enumerate(self.out_names)
        }


_RUNNER = None


def _runner():
    global _RUNNER
    if _RUNNER is None:
        _RUNNER = _Runner(_graph())
    return _RUNNER


def _make_in_maps(X_idx, W, h_pos, J):
    X_idx = np.asarray(X_idx)
    W = np.asarray(W, np.float32)
    jrs = _prep_shared(J, h_pos)
    in_maps = []
    for c in range(NCORES):
        xoht, ohm, wt = _prep_core(
            X_idx[c * MC : (c + 1) * MC], W[c * MC : (c + 1) * MC]
        )
        in_maps.append({"jrs": jrs, "xoht": xoht, "ohm": ohm, "wt": wt})
    return in_maps


def _reg_terms(h_pos, J):
    """Exact float64 regularizers, done on the host (the device computes only
    the data NLL)."""
    J = np.asarray(J, np.float64)
    mask = np.tril(np.ones((L, L)), k=-1)
    regJ = float(np.einsum("ijab,ijab,ij->", J, J, mask))
    h = np.asarray(h_pos, np.float64)
    regH = float((h * h).sum())
    return LAMBDA_J * regJ + LAMBDA_H * regH


# ---------------------------------------------------------------- entry point
def kernel(X_idx, W, h_pos, J):
    in_maps = _make_in_maps(X_idx, W, h_pos, J)
    try:
        r = _runner()
        out = r.run(r.put_inputs(in_maps))["out"]
    except Exception:
        # stock execution path (slower dispatch, same NEFF)
        res = run_bass_kernel_spmd(
            _graph(), in_maps, core_ids=list(range(NCORES))
        )
        out = np.stack([np.asarray(res.results[c]["out"]) for c in range(NCORES)])
    return np.float32(
        np.asarray(out, np.float64).sum() + _reg_terms(h_pos, J)
    )


def bench(X_idx, W, h_pos, J, reps=20):
    """Return (loss, mean_exec_seconds) amortized over reps (incl. RPC)."""
    import time

    r = _runner()
    args = r.put_inputs(_make_in_maps(X_idx, W, h_pos, J))
    out = r.run(args)  # warm-up / compile
    t0 = time.time()
    for _ in range(reps):
        out = r.run(args)
    dt = (time.time() - t0) / reps
    loss = np.asarray(out["out"], np.float64).sum() + _reg_terms(h_pos, J)
    return np.float32(loss), dt

